# revision 1
# baseline (speedup 1.0000x reference)
"""DCGRU cell on 8 Trainium2 NeuronCores (Bass/Tile).

Math: with a = adj + I, d = a.sum(axis=1), T = (d^-1 a)^T,
every diffusion step is  y = T @ v = a^T @ (d_inv * v).
The tensor engine computes lhsT.T @ rhs, so raw adjacency column-slices
feed the PE directly as the stationary operand: no transpose of adj is
ever materialized, and the +I is folded into the per-core slice on host.

Sharding (8 cores): 1D column-parallel over the adjacency.
Core m holds a[:, m*1024:(m+1)*1024] (32 MB fp32, streamed to SBUF once
and kept resident as bf16 for all four diffusion matmuls) and produces
rows m*1024..(m+1)*1024 of every diffusion output. The small [N, 72]
activation is all-gathered (bf16, via HBM bounce buffers) between steps.
Weights/biases are replicated.

Feature order is h-first: j = 0..15 are the hidden/r*h features, j = 16,17
the input features (reference order i is [x(2), h(16)]; host reorders W to
match). Packed node-major column index = b*18 + j. Transposed
(feature-major) tensors are padded to 32 rows per batch (row = b*32 + j) so
every engine access starts at a 0/32/64/96 partition base — a hardware
requirement. With h-first ordering the r*h overwrite lands at base b*32.
"""

import numpy as np
import ml_dtypes

import concourse.bass as bass
import concourse.bacc as bacc
import concourse.tile as tile
import concourse.mybir as mybir
import concourse.bass_utils as bass_utils

F32 = mybir.dt.float32
BF16 = mybir.dt.bfloat16
AF = mybir.ActivationFunctionType
ALU = mybir.AluOpType

NCORES = 8
N = 8192          # nodes
C = N // NCORES   # own nodes per core (1024)
P = 128           # partitions
KCH = N // P      # contraction chunks (64)
MB = C // P       # own-node tiles (8)
B = 4             # batch
IT = 18           # I_tot = in_dim + units
FP = B * IT       # packed feature width (72)
U = 16            # units
IN_DIM = 2

_CACHE = {}


def _build(a_dtype=BF16):
    nc = bacc.Bacc("TRN2", target_bir_lowering=False, debug=False,
                   num_devices=NCORES)

    a_d = nc.dram_tensor("a", [N, C], a_dtype, kind="ExternalInput")
    # p-major (host pre-permuted): element [p, k, f] is node k*128+p
    x0bf_d = nc.dram_tensor("x0bf", [P, KCH, FP], BF16, kind="ExternalInput")
    x0T_d = nc.dram_tensor("x0T", [P, C], BF16, kind="ExternalInput")
    x0own_d = nc.dram_tensor("x0own", [C, FP], F32, kind="ExternalInput")
    hxT_d = nc.dram_tensor("hxT", [P, C], F32, kind="ExternalInput")
    # gate weights, zero-padded to 32 output columns each so every gate psum
    # row gets written (r and u as separate padded matrices)
    wruR_d = nc.dram_tensor("wruR", [P, 3, 32], F32, kind="ExternalInput")
    wruU_d = nc.dram_tensor("wruU", [P, 3, 32], F32, kind="ExternalInput")
    wc_d = nc.dram_tensor("wc", [P, 3, 32], F32, kind="ExternalInput")
    brur_d = nc.dram_tensor("brur", [P, 1], F32, kind="ExternalInput")
    bruu_d = nc.dram_tensor("bruu", [P, 1], F32, kind="ExternalInput")
    bc_d = nc.dram_tensor("bc", [P, 1], F32, kind="ExternalInput")
    ident_d = nc.dram_tensor("ident", [P, P], F32, kind="ExternalInput")
    out_d = nc.dram_tensor("out", [P, C], F32, kind="ExternalOutput")

    with tile.TileContext(nc) as tc:
        with (
            tc.tile_pool(name="big", bufs=1) as big,
            tc.tile_pool(name="zp", bufs=1) as zp,
            tc.tile_pool(name="xtp", bufs=2) as xtp,
            tc.tile_pool(name="psmm", bufs=2, space="PSUM") as psmm,
            tc.tile_pool(name="pstp", bufs=2, space="PSUM") as pstp,
            tc.tile_pool(name="psg", bufs=2, space="PSUM") as psg,
            tc.tile_pool(name="dram", bufs=1, space="DRAM") as dram,
        ):
            # ---------- persistent SBUF tensors ----------
            # resident adjacency slice, one tensor per stream DMA so row-sum
            # reads don't wait on the whole stream (whole-tile dep granularity)
            NDMA = 16
            CPI = KCH // NDMA  # chunks per DMA (1 MB bf16)
            abf_p = [big.tile([P, CPI, C], BF16, name=f"abf{i}")
                     for i in range(NDMA)]

            def abf_chunk(k):
                return abf_p[k // CPI][:, k % CPI, :]
            # row-sum partials: per-half AND per-engine tensors (a shared
            # tensor would serialize DVE and ACT into a cross-engine chain)
            rsAd = big.tile([P, KCH // 4], F32)   # half A, DVE chunks
            rsAa = big.tile([P, KCH // 4], F32)   # half A, ACT chunks
            rsBd = big.tile([P, KCH // 4], F32)
            rsBa = big.tile([P, KCH // 4], F32)
            d_inv = big.tile([P, KCH], F32)
            dsum = big.tile([P, KCH], F32)
            zraw = big.tile([P, KCH, FP], BF16)        # gathered activation (pre-scale)
            x0Tbf = big.tile([P, C], BF16)             # becomes x0cT after rh overwrite
            hxT = big.tile([P, C], F32)                # padded rows b*32+u
            wruR = big.tile([P, 3, 32], F32)
            wruU = big.tile([P, 3, 32], F32)
            wc = big.tile([P, 3, 32], F32)
            wruRbf = big.tile([P, 3, 32], BF16)
            wruUbf = big.tile([P, 3, 32], BF16)
            wcbf = big.tile([P, 3, 32], BF16)
            brur = big.tile([P, 1], F32)
            bruu = big.tile([P, 1], F32)
            bc = big.tile([P, 1], F32)
            identf = big.tile([P, P], F32)
            identbf = big.tile([P, P], BF16)
            x0own = big.tile([P, MB, FP], F32)
            sigR = big.tile([P, C], BF16)              # r gate (rows b*32+u)
            sigU = big.tile([P, C], BF16)              # u gate (rows b*32+u)
            cT = big.tile([P, C], BF16)                # c gate (rows b*32+u)
            outT = big.tile([P, C], F32)               # padded rows b*32+u
            x0cown = big.tile([P, MB, FP], BF16)
            ypack = big.tile([P, MB, FP], BF16)        # packed AG staging
            scratch = big.tile([P, C], BF16)           # gpsimd accum dummy
            # padded-column own-node step outputs (transpose feed); pad cols
            # 18..31 of each 32-block must be zero -> memset once, evacuations
            # only ever write cols 0..17. Two slots shared by (y1p, y1cp) and
            # (x2p, x2cp): pad columns stay zero across reuse.
            y1p = big.tile([P, MB, B, 32], BF16, name="yA")
            x2p = big.tile([P, MB, B, 32], BF16, name="yB")
            y1cp, x2cp = y1p, x2p

            for t in (y1p, x2p):
                nc.vector.memset(t[:], 0.0)

            # early rendezvous: a tiny AllReduce absorbs cross-core launch
            # skew while this core's stream proceeds underneath
            bar_in = dram.tile([P, 1], F32, name="barin")
            bar_out = dram.tile([P, 1], F32, addr_space="Shared", name="barout")
            bar_sb = big.tile([P, 1], F32)
            nc.gpsimd.dma_start(bar_in[:], brur_d[:])
            nc.gpsimd.collective_compute(
                "AllReduce", ALU.add,
                replica_groups=[list(range(NCORES))],
                ins=[bar_in[:]], outs=[bar_out[:]],
            )

            # ---------- phase 0: stream adjacency (bf16) + row sums ----------
            # Per-dma throughput is ~110 GB/s with ~2 outstanding per issuing
            # engine, so spread the stream over three issuers. GpSimd only
            # takes first-half chunks: its later instructions (collective
            # triggers) must not block stream DMA issues.
            # Row sums trail on DVE (pair reduce) + ACT (accum_out copy).
            a_view = a_d[:].rearrange("(k p) c -> p k c", p=P)
            H = KCH // 2
            ENGS = [nc.sync, nc.scalar, nc.gpsimd, nc.sync, nc.scalar,
                    nc.gpsimd, nc.sync, nc.scalar,
                    nc.sync, nc.scalar, nc.sync, nc.scalar,
                    nc.sync, nc.scalar, nc.sync, nc.scalar]
            for it in range(KCH // CPI):
                ENGS[it].dma_start(abf_p[it][:],
                                   a_view[:, it * CPI:(it + 1) * CPI, :])
                # chunk k -> rs column: engine tensors hold quad-interleaved
                # pairs: quad q = (k % H) // 4 covers chunks 4q..4q+3 of the
                # half; DVE gets the first pair, ACT the second.
                for kk in range(0, CPI, 2):
                    k = it * CPI + kk
                    q = (k % H) // 4
                    if kk % 4 == 0:
                        rst = rsAd if k < H else rsBd
                        nc.vector.tensor_reduce(
                            rst[:, 2 * q:2 * q + 2],
                            abf_p[it][:, kk:kk + 2, :],
                            axis=mybir.AxisListType.X, op=ALU.add,
                        )
                    else:
                        rst = rsAa if k < H else rsBa
                        for j, k2 in enumerate((k, k + 1)):
                            nc.scalar.activation(
                                scratch[:], abf_chunk(k2), AF.Copy,
                                accum_out=rst[:, 2 * q + j:2 * q + j + 1],
                            )

                # issue the first half-AllReduce as soon as the first half of
                # the stream is in flight (DMAs on vector/gpsimd so engine
                # program order can't delay them behind stream DMAs)
                if it == KCH // CPI // 2 - 1:
                    rs_in0 = dram.tile([P, H], F32, name="rsin0")
                    rs_out0 = dram.tile([P, H], F32, addr_space="Shared",
                                        name="rsout0")
                    riv = rs_in0[:].rearrange("p (q e) -> p q e", e=4)
                    nc.gpsimd.dma_start(
                        riv[:, :, 0:2],
                        rsAd[:].rearrange("p (q e) -> p q e", e=2))
                    nc.gpsimd.dma_start(
                        riv[:, :, 2:4],
                        rsAa[:].rearrange("p (q e) -> p q e", e=2))
                    nc.gpsimd.collective_compute(
                        "AllReduce", ALU.add,
                        replica_groups=[list(range(NCORES))],
                        ins=[rs_in0[:]], outs=[rs_out0[:]],
                    )
                    nc.scalar.dma_start(dsum[:, 0:H], rs_out0[:])

            # ---------- small input DMAs (issue behind the stream) ----------
            nc.scalar.dma_start(zraw[:], x0bf_d[:])
            nc.sync.dma_start(
                x0own[:], x0own_d[:].rearrange("(m p) f -> p m f", p=P))
            nc.sync.dma_start(x0Tbf[:], x0T_d[:])
            nc.sync.dma_start(hxT[:], hxT_d[:])
            nc.sync.dma_start(wruR[:], wruR_d[:])
            nc.sync.dma_start(wruU[:], wruU_d[:])
            nc.sync.dma_start(wc[:], wc_d[:])
            nc.sync.dma_start(brur[:], brur_d[:])
            nc.sync.dma_start(bruu[:], bruu_d[:])
            nc.sync.dma_start(bc[:], bc_d[:])
            nc.sync.dma_start(identf[:], ident_d[:])
            nc.vector.tensor_copy(wruRbf[:], wruR[:])
            nc.vector.tensor_copy(wruUbf[:], wruU[:])
            nc.vector.tensor_copy(wcbf[:], wc[:])
            nc.vector.tensor_copy(identbf[:], identf[:])
            nc.scalar.dma_start(bar_sb[:], bar_out[:])

            # ---------- phase 1: second half-AllReduce ----------
            rs_in1 = dram.tile([P, H], F32, name="rsin1")
            rs_out1 = dram.tile([P, H], F32, addr_space="Shared", name="rsout1")
            riv1 = rs_in1[:].rearrange("p (q e) -> p q e", e=4)
            nc.gpsimd.dma_start(
                riv1[:, :, 0:2], rsBd[:].rearrange("p (q e) -> p q e", e=2))
            nc.gpsimd.dma_start(
                riv1[:, :, 2:4], rsBa[:].rearrange("p (q e) -> p q e", e=2))
            nc.gpsimd.collective_compute(
                "AllReduce", ALU.add,
                replica_groups=[list(range(NCORES))],
                ins=[rs_in1[:]], outs=[rs_out1[:]],
            )
            nc.scalar.dma_start(dsum[:, H:], rs_out1[:])
            nc.vector.reciprocal(d_inv[:, 0:H], dsum[:, 0:H])
            nc.vector.reciprocal(d_inv[:, H:], dsum[:, H:])

            dinv_b = d_inv[:].unsqueeze(-1).broadcast_to((P, KCH, FP))

            SPL = 52  # DVE/GpSimd work split for the z scaling (~4x speed ratio)
            dinv_bh = [
                d_inv[:, 0:SPL].unsqueeze(-1).broadcast_to((P, SPL, FP)),
                d_inv[:, SPL:].unsqueeze(-1).broadcast_to((P, KCH - SPL, FP)),
            ]

            def scale(name):
                z = zp.tile([P, KCH, FP], BF16, tag="z", name=name)
                nc.vector.tensor_tensor(
                    z[:, 0:SPL, :], zraw[:, 0:SPL, :], dinv_bh[0], ALU.mult)
                nc.gpsimd.tensor_tensor(
                    z[:, SPL:, :], zraw[:, SPL:, :], dinv_bh[1], ALU.mult)
                return z

            def diffusion(z, dst, combine=None):
                """dst[:, mb, b, 0:18] = a^T-slice @ z (+ combine hook)."""
                for mb in range(MB):
                    ps = psmm.tile([P, FP], F32, tag="mm")
                    for k in range(KCH):
                        nc.tensor.matmul(
                            ps[:],
                            lhsT=abf_chunk(k)[:, mb * P:(mb + 1) * P],
                            rhs=z[:, k, :],
                            start=(k == 0), stop=(k == KCH - 1),
                        )
                    psv = ps[:].rearrange("p (b i) -> p b i", b=B)
                    dstv = dst[:, mb, :, 0:IT]
                    if combine is None:
                        nc.scalar.activation(dstv, psv, AF.Copy)
                    else:
                        combine(mb, dstv, psv)
                return dst

            def allgather(src_packed):
                """Gather packed own tiles [P, MB, FP] -> zraw (bf16).
                p-major bounce layout keeps both DMAs contiguous per
                partition; gathered chunk (r, m) is exactly k = r*MB + m."""
                agin = dram.tile([P, MB, FP], BF16, tag="agin")
                agout = dram.tile([NCORES, P, MB, FP], BF16,
                                  addr_space="Shared", tag="agout")
                nc.sync.dma_start(agin[:], src_packed[:])
                nc.gpsimd.collective_compute(
                    "AllGather", ALU.bypass,
                    replica_groups=[list(range(NCORES))],
                    ins=[agin[:]], outs=[agout[:]],
                )
                nc.sync.dma_start(
                    zraw[:].rearrange("p (r m) f -> p r (m f)", r=NCORES),
                    agout[:].rearrange("r p m f -> p r (m f)"),
                )

            def repack(src_p):
                """Padded [P, MB, B, 32] -> packed ypack [P, MB, FP]."""
                nc.vector.tensor_copy(
                    ypack[:].rearrange("p m (b i) -> p m b i", b=B),
                    src_p[:, :, :, 0:IT],
                )
                return ypack

            def transposes(src_p, xT_name):
                """src_p [P, MB, B, 32] (padded bf16) -> xT [P, C] feature-major."""
                xT = xtp.tile([P, C], BF16, tag="xT", name=xT_name)
                for mb in range(MB):
                    pt = pstp.tile([P, P], BF16, tag="tp")
                    nc.tensor.transpose(
                        pt[:],
                        src_p[:, mb, :, :].rearrange("p b i -> p (b i)"),
                        identbf[:],
                    )
                    if mb % 2 == 0:
                        nc.vector.tensor_copy(xT[:, mb * P:(mb + 1) * P], pt[:])
                    else:
                        nc.scalar.activation(xT[:, mb * P:(mb + 1) * P], pt[:],
                                             AF.Copy)
                return xT

            def gate_matmuls(wbf, m_srcs, gname):
                """psum[b*32:(b+1)*32, :] = sum_m W_m^T @ x_mT per b (M=32,
                zero-padded weight cols; every psum row gets written)."""
                halves = []
                for h in range(2):
                    pg = psg.tile([P, C // 2], F32, tag="gate",
                                  name=f"pg{gname}{h}", bufs=4)
                    for b in range(B):
                        for m in range(3):
                            nc.tensor.matmul(
                                pg[b * 32:(b + 1) * 32, :],
                                lhsT=wbf[b * 32:b * 32 + IT, m, :],
                                rhs=m_srcs[m][b * 32:b * 32 + IT,
                                              h * (C // 2):(h + 1) * (C // 2)],
                                start=(m == 0), stop=(m == 2),
                                tile_position=(b * 32, b * 32),
                            )
                    halves.append(pg)
                return halves

            def act_full(dst, pgs, func, bias):
                for h in range(2):
                    nc.scalar.activation(
                        dst[:, h * (C // 2):(h + 1) * (C // 2)], pgs[h][:],
                        func, bias=bias[:],
                    )

            # ================= gconv 1 (r/u gates) =================
            z0 = scale("z0")
            diffusion(z0, y1p)
            allgather(repack(y1p))              # -> zraw = x1 (full, bf16)
            x1T = transposes(y1p, "x1T")        # overlaps the AllGather
            z1 = scale("z1")

            def comb_b(mb, dstv, psv):
                # x2 = 2*psum - x0   (own rows)
                x0v = x0own[:, mb, :].rearrange("p (b i) -> p b i", b=B)
                nc.vector.tensor_scalar_mul(dstv, psv, 2.0)
                nc.vector.tensor_tensor(dstv, dstv, x0v, ALU.subtract)

            diffusion(z1, x2p, combine=comb_b)
            x2T = transposes(x2p, "x2T")

            pg_r = gate_matmuls(wruRbf, [x0Tbf, x1T, x2T], "r")
            pg_u = gate_matmuls(wruUbf, [x0Tbf, x1T, x2T], "u")
            act_full(sigR, pg_r, AF.Sigmoid, brur)
            act_full(sigU, pg_u, AF.Sigmoid, bruu)

            # rh overwrites the h-rows (b*32+0..15) of x0Tbf -> becomes x0cT
            for b in range(B):
                nc.vector.tensor_tensor(
                    x0Tbf[b * 32:b * 32 + U, :],
                    sigR[b * 32:b * 32 + U, :],
                    hxT[b * 32:b * 32 + U, :],
                    ALU.mult,
                )

            # x0c own-node tiles (node-major) via transpose of x0cT
            for mb in range(MB):
                pt = pstp.tile([P, P], BF16, tag="tp")
                nc.tensor.transpose(pt[:], x0Tbf[:, mb * P:(mb + 1) * P], identbf[:])
                srcv = pt[:].rearrange("p (b e) -> p b e", b=B)[:, :, 0:IT]
                dstv = x0cown[:, mb, :].rearrange("p (b i) -> p b i", b=B)
                if mb % 2 == 0:
                    nc.vector.tensor_copy(dstv, srcv)
                else:
                    nc.scalar.activation(dstv, srcv, AF.Copy)

            # gather x0c (rh part changed) and scale
            allgather(x0cown)

            # ================= gconv 2 (candidate c) =================
            z0c = scale("z0c")
            diffusion(z0c, y1cp)
            allgather(repack(y1cp))
            x1cT = transposes(y1cp, "x1cT")
            z1c = scale("z1c")

            def comb_d(mb, dstv, psv):
                x0cv = x0cown[:, mb, :].rearrange("p (b i) -> p b i", b=B)
                nc.vector.tensor_scalar_mul(dstv, psv, 2.0)
                nc.vector.tensor_tensor(dstv, dstv, x0cv, ALU.subtract)

            diffusion(z1c, x2cp, combine=comb_d)
            x2cT = transposes(x2cp, "x2cT")

            pg_c = gate_matmuls(wcbf, [x0Tbf, x1cT, x2cT], "c")
            act_full(cT, pg_c, AF.Tanh, bc)

            # out = u*h + (1-u)*c = c + u*(h-c), in place in outT, full
            # 128-partition halves split across DVE and GpSimd
            for h, eng in ((0, nc.vector), (1, nc.gpsimd)):
                fs = slice(h * (C // 2), (h + 1) * (C // 2))
                eng.tensor_tensor(outT[:, fs], hxT[:, fs], cT[:, fs], ALU.subtract)
                eng.tensor_tensor(outT[:, fs], outT[:, fs], sigU[:, fs], ALU.mult)
                eng.tensor_tensor(outT[:, fs], outT[:, fs], cT[:, fs], ALU.add)

            # ship feature-major [b*32+u, n]; host undoes the layout
            nc.sync.dma_start(out_d[:], outT[:])

    nc.compile()
    return nc


def _get_nc():
    if "nc" not in _CACHE:
        _CACHE["nc"] = _build()
    return _CACHE["nc"]


# feature permutation: device feature j -> reference feature i
# j = 0..15 -> i = j+2 (hidden), j = 16,17 -> i = j-16 (input x)
_PERM = np.array(list(range(2, 18)) + [0, 1])


def _host_prep(inputs, hx, adj, W_ru, b_ru, W_c, b_c):
    f32 = np.float32
    xr = np.ascontiguousarray(inputs, dtype=f32).reshape(B, N, IN_DIM)
    hr = np.ascontiguousarray(hx, dtype=f32).reshape(B, N, U)
    x0 = np.concatenate([hr, xr], axis=2).transpose(1, 0, 2)  # [N, B, 18] h-first
    x0 = np.ascontiguousarray(x0).reshape(N, FP)
    # p-major: [p, k, f] = node k*128+p
    x0bf = np.ascontiguousarray(
        x0.astype(ml_dtypes.bfloat16).reshape(KCH, P, FP).transpose(1, 0, 2))

    def pad_w(w, lo):
        # w rows (i, m) reference-ordered -> [P, 3, 32] padded both ways
        w3 = np.asarray(w, f32).reshape(IT, 3, -1)[_PERM][:, :, lo:lo + U]
        out = np.zeros((B, 32, 3, 32), f32)
        out[:, 0:IT, :, 0:U] = w3[None]
        return out.reshape(P, 3, 32)

    wruR_p = pad_w(W_ru, 0)
    wruU_p = pad_w(W_ru, U)
    wc_p = pad_w(W_c, 0)

    def pad_bias(v):
        t = np.zeros((B, 32), f32)
        t[:, 0:U] = np.asarray(v, f32)
        return np.ascontiguousarray(t.reshape(P)[:, None])

    brur_t = pad_bias(np.asarray(b_ru, f32)[0:U])
    bruu_t = pad_bias(np.asarray(b_ru, f32)[U:2 * U])
    bc_t = pad_bias(np.asarray(b_c, f32))
    ident = np.eye(P, dtype=f32)

    adj = np.asarray(adj, f32)
    in_maps = []
    for m in range(NCORES):
        sl = slice(m * C, (m + 1) * C)
        a_m = np.ascontiguousarray(adj[:, sl])
        a_m[m * C + np.arange(C), np.arange(C)] += 1.0
        a_m = a_m.astype(ml_dtypes.bfloat16)
        x0own = np.ascontiguousarray(x0[sl])
        x0T = np.zeros((B, 32, C), f32)
        x0T[:, 0:IT, :] = x0own.reshape(C, B, IT).transpose(1, 2, 0)
        x0T = x0T.reshape(P, C).astype(ml_dtypes.bfloat16)
        hxT_p = np.zeros((B, 32, C), f32)
        hxT_p[:, 0:U, :] = hr[:, sl, :].transpose(0, 2, 1)
        hxT_p = hxT_p.reshape(P, C)
        in_maps.append({
            "a": a_m,
            "x0bf": x0bf,
            "x0T": x0T,
            "x0own": x0own,
            "hxT": hxT_p,
            "wruR": wruR_p,
            "wruU": wruU_p,
            "wc": wc_p,
            "brur": brur_t,
            "bruu": bruu_t,
            "bc": bc_t,
            "ident": ident,
        })
    return in_maps


def _run(in_maps, trace=False, **kw):
    nc = _get_nc()
    return bass_utils.run_bass_kernel_spmd(
        nc, in_maps, core_ids=list(range(NCORES)), trace=trace, **kw)


def _assemble(results):
    out = np.empty((B, N * U), np.float32)
    for m in range(NCORES):
        # device layout [b*32+u, n] (rows 16..31 per block are padding)
        blk = results[m]["out"].reshape(B, 32, C)[:, 0:U, :].transpose(0, 2, 1)
        out[:, m * C * U:(m + 1) * C * U] = blk.reshape(B, C * U)
    return out


def kernel(inputs, hx, adj, W_ru, b_ru, W_c, b_c):
    in_maps = _host_prep(inputs, hx, adj, W_ru, b_ru, W_c, b_c)
    res = _run(in_maps)
    return _assemble(res.results)



# revision 3
# speedup vs baseline: 1.5497x; 1.5497x over previous
"""DCGRU cell on 8 Trainium2 NeuronCores (Bass/Tile), v2.

Math: with a = adj + I, d = a.sum(axis=1), T = (d^-1 a)^T,
every diffusion step is  y = T @ v = a^T @ (d_inv * v).
d_inv is computed on the HOST (it only depends on adj), which removes
the row-sum AllReduces of v1 entirely.  The d_inv factor rides on the
activation side: the stationary operand of each diffusion matmul is
z = c * d_inv * v (c a power-of-2 so fp8 values sit in normal range),
and the 1/c unscale folds into the PSUM evacuation.

Sharding (8 cores): 1D column-parallel over the adjacency.  Core m
holds a[:, m*1024:(m+1)*1024] as fp8e4 (8 MB), host-permuted p-major so
every partition line is contiguous in DRAM.  Each diffusion is a
DoubleRow fp8 matmul: stationary z chunk [128, 2, 128] (two node-chunks
deep), moving adjacency [128, 2, 512] -> psum [128, 512].  The OUTPUT
is feature-major [b*32+j, own-node] which is exactly the layout the
gate matmuls need, so gate inputs need no transposes; only the
AllGather payloads (node-major) need 8 PE transposes each.

Feature order is h-first: j = 0..15 hidden, j = 16,17 input; padded to
32 rows per batch (row b*32+j) so every partition base is 0/32/64/96.
Packed payload index = b*18 + j.
"""

import numpy as np
import ml_dtypes

import concourse.bass as bass
import concourse.bacc as bacc
import concourse.tile as tile
import concourse.mybir as mybir
import concourse.bass_utils as bass_utils

F32 = mybir.dt.float32
BF16 = mybir.dt.bfloat16
FP8 = mybir.dt.float8e4
AF = mybir.ActivationFunctionType
ALU = mybir.AluOpType
DR = mybir.MatmulPerfMode.DoubleRow

NCORES = 8
N = 8192          # nodes
C = N // NCORES   # own nodes per core (1024)
P = 128           # partitions
KCH = N // P      # node chunks (64)
KP = KCH // 2     # DoubleRow chunk pairs (32)
MB = C // P       # own-node tiles (8)
B = 4             # batch
IT = 18           # I_tot = in_dim + units
FP = B * IT       # packed feature width (72)
FPAD = B * 32     # padded feature width (128)
U = 16            # units
IN_DIM = 2
HC = C // 2       # half own-node width (512)

C0 = 4096.0       # z0 = C0 * d_inv * x0     (fp8-range normalizer)
C1 = 262144.0     # z1 = C1 * d_inv * x1
S0 = 1.0 / C0     # psum -> x1 unscale
S1 = 2.0 / C1     # psum -> 2*T@x1 unscale

_CACHE = {}


def _build():
    nc = bacc.Bacc("TRN2", target_bir_lowering=False, debug=False,
                   num_devices=NCORES)

    a_d = nc.dram_tensor("a", [P, KCH, C], FP8, kind="ExternalInput")
    z0_d = nc.dram_tensor("z0", [P, KCH, FPAD], FP8, kind="ExternalInput")
    x0T_d = nc.dram_tensor("x0T", [P, C], BF16, kind="ExternalInput")
    hxT_d = nc.dram_tensor("hxT", [P, C], BF16, kind="ExternalInput")
    wruR_d = nc.dram_tensor("wruR", [P, 3, 32], BF16, kind="ExternalInput")
    wruU_d = nc.dram_tensor("wruU", [P, 3, 32], BF16, kind="ExternalInput")
    wc_d = nc.dram_tensor("wc", [P, 3, 32], BF16, kind="ExternalInput")
    brur_d = nc.dram_tensor("brur", [P, 1], F32, kind="ExternalInput")
    bruu_d = nc.dram_tensor("bruu", [P, 1], F32, kind="ExternalInput")
    bc_d = nc.dram_tensor("bc", [P, 1], F32, kind="ExternalInput")
    cd0_d = nc.dram_tensor("cd0", [P, KCH], BF16, kind="ExternalInput")
    cd1_d = nc.dram_tensor("cd1", [P, KCH], BF16, kind="ExternalInput")
    ident_d = nc.dram_tensor("ident", [P, P], BF16, kind="ExternalInput")
    out_d = nc.dram_tensor("out", [P, C], F32, kind="ExternalOutput")

    with tile.TileContext(nc) as tc:
        with (
            tc.tile_pool(name="big", bufs=1) as big,
            tc.tile_pool(name="psmm", bufs=2, space="PSUM") as psmm,
            tc.tile_pool(name="pstp", bufs=2, space="PSUM") as pstp,
            tc.tile_pool(name="psg", bufs=4, space="PSUM") as psg,
            tc.tile_pool(name="dram", bufs=1, space="DRAM") as dram,
        ):
            # ---------- persistent SBUF tensors ----------
            NDMA = 16
            CPI = KCH // NDMA  # chunks per stream DMA (4)
            abf = [big.tile([P, CPI, C], FP8, name=f"abf{i}")
                   for i in range(NDMA)]

            z0 = big.tile([P, KCH, FPAD], FP8)
            z1 = big.tile([P, KCH, FPAD], FP8)
            z0c = big.tile([P, KCH, FPAD], FP8)
            z1c = big.tile([P, KCH, FPAD], FP8)
            g1raw = big.tile([P, KCH, FP], BF16)   # gathered x1
            g0craw = big.tile([P, KCH, FP], BF16)  # gathered x0c
            g1craw = big.tile([P, KCH, FP], BF16)  # gathered x1c
            x0T = big.tile([P, C], BF16)           # becomes x0cT after rh
            hxT = big.tile([P, C], BF16)
            y1T = big.tile([P, C], BF16)
            x2T = big.tile([P, C], BF16)
            y1cT = big.tile([P, C], BF16)
            x2cT = big.tile([P, C], BF16)
            sigR = big.tile([P, C], BF16)
            sigU = big.tile([P, C], BF16)
            cT = big.tile([P, C], BF16)
            outT = big.tile([P, C], F32)
            wruR = big.tile([P, 3, 32], BF16)
            wruU = big.tile([P, 3, 32], BF16)
            wc = big.tile([P, 3, 32], BF16)
            brur = big.tile([P, 1], F32)
            bruu = big.tile([P, 1], F32)
            bc = big.tile([P, 1], F32)
            cd0 = big.tile([P, KCH], BF16)
            cd1 = big.tile([P, KCH], BF16)
            identbf = big.tile([P, P], BF16)
            pk1 = big.tile([P, MB, FP], BF16)      # AG payload staging
            pk0c = big.tile([P, MB, FP], BF16)
            pk1c = big.tile([P, MB, FP], BF16)

            # z pad columns (j=18..31 per batch block) must stay zero so
            # diffusion pad partitions come out zero; builds only write 0:18
            for t in (z1, z0c, z1c):
                nc.vector.memset(t[:], 0.0)

            # ---------- input DMAs ----------
            nc.sync.dma_start(z0[:], z0_d[:])
            nc.scalar.dma_start(x0T[:], x0T_d[:])
            nc.scalar.dma_start(hxT[:], hxT_d[:])
            nc.gpsimd.dma_start(wruR[:], wruR_d[:])
            nc.gpsimd.dma_start(wruU[:], wruU_d[:])
            nc.gpsimd.dma_start(wc[:], wc_d[:])
            nc.gpsimd.dma_start(brur[:], brur_d[:])
            nc.gpsimd.dma_start(bruu[:], bruu_d[:])
            nc.gpsimd.dma_start(bc[:], bc_d[:])
            nc.gpsimd.dma_start(cd0[:], cd0_d[:])
            nc.gpsimd.dma_start(cd1[:], cd1_d[:])
            nc.gpsimd.dma_start(identbf[:], ident_d[:])

            # ---------- adjacency stream (fp8, p-major contiguous) ----------
            ENGS = ([nc.sync, nc.scalar, nc.gpsimd] * 6)[:NDMA]
            for i in range(NDMA):
                ENGS[i].dma_start(abf[i][:], a_d[:, i * CPI:(i + 1) * CPI, :])

            def apair(kp, h):
                """Moving operand [128, 2, 512] for chunk pair kp, half h."""
                i, kk = (2 * kp) // CPI, (2 * kp) % CPI
                return abf[i][:, kk:kk + 2, h * HC:(h + 1) * HC]

            def diffusion(z, tag):
                """psum halves [128, 512] = sum_k z_k^T(a-chunk), DoubleRow."""
                ps = [psmm.tile([P, HC], F32, tag="mm", name=f"ps{tag}{h}")
                      for h in range(2)]
                for kp in range(KP):
                    lz = z[:, 2 * kp:2 * kp + 2, :]
                    for h in range(2):
                        nc.tensor.matmul(
                            ps[h][:], lhsT=lz, rhs=apair(kp, h),
                            start=(kp == 0), stop=(kp == KP - 1),
                            perf_mode=DR,
                        )
                return ps

            def evac(dst, ps, scale):
                """psum -> bf16 feature-major, ACT half + DVE half."""
                nc.scalar.activation(dst[:, 0:HC], ps[0][:], AF.Copy,
                                     scale=scale)
                nc.vector.tensor_scalar_mul(dst[:, HC:], ps[1][:], scale)

            def to_packed(srcT, pk):
                """Feature-major [P, C] -> packed node-major [P, MB, FP]."""
                for mb in range(MB):
                    pt = pstp.tile([P, P], BF16, tag="tp")
                    nc.tensor.transpose(
                        pt[:], srcT[:, mb * P:(mb + 1) * P], identbf[:])
                    srcv = pt[:].rearrange("p (b e) -> p b e", b=B)[:, :, 0:IT]
                    dstv = pk[:, mb, :].rearrange("p (b i) -> p b i", b=B)
                    if mb % 2 == 0:
                        nc.vector.tensor_copy(dstv, srcv)
                    else:
                        nc.scalar.activation(dstv, srcv, AF.Copy)

            def allgather(pk, graw, tag):
                agin = dram.tile([P, MB, FP], BF16, tag="agin")
                agout = dram.tile([NCORES, P, MB, FP], BF16,
                                  addr_space="Shared", tag="agout")
                nc.sync.dma_start(agin[:], pk[:])
                nc.gpsimd.collective_compute(
                    "AllGather", ALU.bypass,
                    replica_groups=[list(range(NCORES))],
                    ins=[agin[:]], outs=[agout[:]],
                )
                nc.sync.dma_start(
                    graw[:].rearrange("p (r m) f -> p r (m f)", r=NCORES),
                    agout[:].rearrange("r p m f -> p r (m f)"),
                )

            def zbuild(z, graw, cd):
                """z[:, :, b*32+0:18] = cd * gathered  (one DVE op, fp8 out)."""
                dst = z[:].rearrange("p k (b e) -> p k b e", b=B)[:, :, :, 0:IT]
                src = graw[:].rearrange("p k (b i) -> p k b i", b=B)
                cdb = cd[:].unsqueeze(-1).unsqueeze(-1).broadcast_to(
                    (P, KCH, B, IT))
                nc.vector.tensor_tensor(dst, src, cdb, ALU.mult)

            def gate_matmuls(wbf, m_srcs, gname):
                halves = []
                for h in range(2):
                    pg = psg.tile([P, HC], F32, tag="gate",
                                  name=f"pg{gname}{h}", bufs=4)
                    for b in range(B):
                        for m in range(3):
                            nc.tensor.matmul(
                                pg[b * 32:(b + 1) * 32, :],
                                lhsT=wbf[b * 32:b * 32 + IT, m, :],
                                rhs=m_srcs[m][b * 32:b * 32 + IT,
                                              h * HC:(h + 1) * HC],
                                start=(m == 0), stop=(m == 2),
                                tile_position=(b * 32, b * 32),
                            )
                    halves.append(pg)
                return halves

            def act_full(dst, pgs, func, bias):
                for h in range(2):
                    nc.scalar.activation(
                        dst[:, h * HC:(h + 1) * HC], pgs[h][:], func,
                        bias=bias[:],
                    )

            # ================= gconv 1 (r/u gates) =================
            psA = diffusion(z0, "A")               # chases the stream
            evac(y1T, psA, S0)                     # y1T = x1 own
            to_packed(y1T, pk1)
            allgather(pk1, g1raw, "x1")
            zbuild(z1, g1raw, cd1)

            psB = diffusion(z1, "B")
            # x2 = 2*T@x1 - x0  (psB = C1/2 * that first term)
            for h in range(2):
                fs = slice(h * HC, (h + 1) * HC)
                nc.vector.tensor_scalar_mul(x2T[:, fs], psB[h][:], S1)
                nc.vector.tensor_tensor(x2T[:, fs], x2T[:, fs], x0T[:, fs],
                                        ALU.subtract)

            pg_r = gate_matmuls(wruR, [x0T, y1T, x2T], "r")
            pg_u = gate_matmuls(wruU, [x0T, y1T, x2T], "u")
            act_full(sigR, pg_r, AF.Sigmoid, brur)

            # rh overwrites h-rows (b*32+0..15) of x0T -> becomes x0cT
            for b in range(B):
                nc.vector.tensor_tensor(
                    x0T[b * 32:b * 32 + U, :],
                    sigR[b * 32:b * 32 + U, :],
                    hxT[b * 32:b * 32 + U, :],
                    ALU.mult,
                )
            to_packed(x0T, pk0c)
            allgather(pk0c, g0craw, "x0c")
            act_full(sigU, pg_u, AF.Sigmoid, bruu)   # fills the AG window

            # ================= gconv 2 (candidate c) =================
            zbuild(z0c, g0craw, cd0)
            psC = diffusion(z0c, "C")
            evac(y1cT, psC, S0)
            to_packed(y1cT, pk1c)
            allgather(pk1c, g1craw, "x1c")
            zbuild(z1c, g1craw, cd1)

            psD = diffusion(z1c, "D")
            for h in range(2):
                fs = slice(h * HC, (h + 1) * HC)
                nc.vector.tensor_scalar_mul(x2cT[:, fs], psD[h][:], S1)
                nc.vector.tensor_tensor(x2cT[:, fs], x2cT[:, fs], x0T[:, fs],
                                        ALU.subtract)

            pg_c = gate_matmuls(wc, [x0T, y1cT, x2cT], "c")
            act_full(cT, pg_c, AF.Tanh, bc)

            # out = u*h + (1-u)*c = c + u*(h-c)
            for h in range(2):
                fs = slice(h * HC, (h + 1) * HC)
                nc.vector.tensor_tensor(outT[:, fs], hxT[:, fs], cT[:, fs],
                                        ALU.subtract)
                nc.vector.tensor_tensor(outT[:, fs], outT[:, fs], sigU[:, fs],
                                        ALU.mult)
                nc.vector.tensor_tensor(outT[:, fs], outT[:, fs], cT[:, fs],
                                        ALU.add)

            nc.sync.dma_start(out_d[:], outT[:])

    nc.compile()
    return nc


def _get_nc():
    if "nc" not in _CACHE:
        _CACHE["nc"] = _build()
    return _CACHE["nc"]


# feature permutation: device feature j -> reference feature i
# j = 0..15 -> i = j+2 (hidden), j = 16,17 -> i = j-16 (input x)
_PERM = np.array(list(range(2, 18)) + [0, 1])


def _host_prep(inputs, hx, adj, W_ru, b_ru, W_c, b_c):
    f32 = np.float32
    bf16 = ml_dtypes.bfloat16
    fp8 = ml_dtypes.float8_e4m3fn

    xr = np.ascontiguousarray(inputs, dtype=f32).reshape(B, N, IN_DIM)
    hr = np.ascontiguousarray(hx, dtype=f32).reshape(B, N, U)
    x0 = np.concatenate([hr, xr], axis=2).transpose(1, 0, 2)  # [N, B, 18]
    x0 = np.ascontiguousarray(x0).reshape(N, FP)

    adj = np.asarray(adj, f32)
    d = adj.sum(axis=1) + 1.0
    dinv = 1.0 / d

    # z0 = C0 * dinv * x0, padded to 32 cols per batch, p-major, fp8
    z0 = np.zeros((N, B, 32), f32)
    z0[:, :, 0:IT] = (C0 * dinv)[:, None, None] * x0.reshape(N, B, IT)
    z0 = z0.reshape(KCH, P, FPAD).transpose(1, 0, 2)  # [P, KCH, FPAD]
    z0 = np.ascontiguousarray(z0).astype(fp8)

    cdv = dinv.reshape(KCH, P).T  # [P, KCH]
    cd0 = np.ascontiguousarray(C0 * cdv).astype(bf16)
    cd1 = np.ascontiguousarray(C1 * cdv).astype(bf16)

    def pad_w(w, lo):
        w3 = np.asarray(w, f32).reshape(IT, 3, -1)[_PERM][:, :, lo:lo + U]
        out = np.zeros((B, 32, 3, 32), f32)
        out[:, 0:IT, :, 0:U] = w3[None]
        return out.reshape(P, 3, 32).astype(bf16)

    wruR_p = pad_w(W_ru, 0)
    wruU_p = pad_w(W_ru, U)
    wc_p = pad_w(W_c, 0)

    def pad_bias(v):
        t = np.zeros((B, 32), f32)
        t[:, 0:U] = np.asarray(v, f32)
        return np.ascontiguousarray(t.reshape(P)[:, None])

    brur_t = pad_bias(np.asarray(b_ru, f32)[0:U])
    bruu_t = pad_bias(np.asarray(b_ru, f32)[U:2 * U])
    bc_t = pad_bias(np.asarray(b_c, f32))
    ident = np.eye(P, dtype=f32).astype(bf16)

    in_maps = []
    for m in range(NCORES):
        sl = slice(m * C, (m + 1) * C)
        a_m = np.ascontiguousarray(adj[:, sl])
        a_m[m * C + np.arange(C), np.arange(C)] += 1.0
        # p-major: [p, k, c] = row k*128+p
        a_m = a_m.reshape(KCH, P, C).transpose(1, 0, 2)
        a_m = np.ascontiguousarray(a_m).astype(fp8)

        x0own = x0[sl]
        x0T = np.zeros((B, 32, C), f32)
        x0T[:, 0:IT, :] = x0own.reshape(C, B, IT).transpose(1, 2, 0)
        x0T = x0T.reshape(P, C).astype(bf16)
        hxT_p = np.zeros((B, 32, C), f32)
        hxT_p[:, 0:U, :] = hr[:, sl, :].transpose(0, 2, 1)
        hxT_p = hxT_p.reshape(P, C).astype(bf16)
        in_maps.append({
            "a": a_m,
            "z0": z0,
            "x0T": x0T,
            "hxT": hxT_p,
            "wruR": wruR_p,
            "wruU": wruU_p,
            "wc": wc_p,
            "brur": brur_t,
            "bruu": bruu_t,
            "bc": bc_t,
            "cd0": cd0,
            "cd1": cd1,
            "ident": ident,
        })
    return in_maps


def _run(in_maps, trace=False, **kw):
    nc = _get_nc()
    return bass_utils.run_bass_kernel_spmd(
        nc, in_maps, core_ids=list(range(NCORES)), trace=trace, **kw)


def _assemble(results):
    out = np.empty((B, N * U), np.float32)
    for m in range(NCORES):
        # device layout [b*32+u, n] (rows 16..31 per block are padding)
        blk = results[m]["out"].reshape(B, 32, C)[:, 0:U, :].transpose(0, 2, 1)
        out[:, m * C * U:(m + 1) * C * U] = blk.reshape(B, C * U)
    return out


def kernel(inputs, hx, adj, W_ru, b_ru, W_c, b_c):
    in_maps = _host_prep(inputs, hx, adj, W_ru, b_ru, W_c, b_c)
    res = _run(in_maps)
    return _assemble(res.results)


# revision 5
# speedup vs baseline: 1.6584x; 1.0702x over previous
"""DCGRU cell on 8 Trainium2 NeuronCores (Bass/Tile), v3.

Math: with a = adj + I, d = a.sum(axis=1), T = (d^-1 a)^T, every
diffusion step is  y = T @ v = a^T @ (d_inv * v).  d_inv is computed on
the HOST, so there are no row-sum collectives.  The d_inv factor rides
on the activation side: the stationary operand of each diffusion matmul
is z = c * d_inv * v (c a power of 2 keeping fp8 values in normal
range).  All unscale constants fold into the host-prepared gate weights
(W0' = W0 - W2, W1' = W1/c0, W2' = 2*W2/c1), so diffusion PSUMs are
evacuated as raw bf16 copies and the Chebyshev combine x2 = 2*T@x1 - x0
never materializes.

Sharding (8 cores): 1D column-parallel over the adjacency.  Core m
holds a[:, m*1024:(m+1)*1024] as fp8e4 (8 MB), host-permuted p-major so
partition lines are contiguous.  Each diffusion is a DoubleRow fp8
matmul: stationary z pair-chunk [128, 2, 128], moving adjacency
[128, 2, 512] -> psum [128, 512]; output is feature-major [b*32+j, n]
which is what the gate matmuls consume directly.  AllGather payloads
are node-major padded fp8, pre-scaled by c*d_inv during the
transpose-evacuation, so the gather DMA writes the next diffusion's
stationary operand directly (no on-device z build at all).

Feature order is h-first: j = 0..15 hidden, j = 16,17 input; padded to
32 rows/cols per batch so partition bases stay 0/32/64/96.
"""

import numpy as np
import ml_dtypes

import concourse.bass as bass
import concourse.bacc as bacc
import concourse.tile as tile
import concourse.mybir as mybir
import concourse.bass_utils as bass_utils

F32 = mybir.dt.float32
BF16 = mybir.dt.bfloat16
FP8 = mybir.dt.float8e4
AF = mybir.ActivationFunctionType
ALU = mybir.AluOpType
DR = mybir.MatmulPerfMode.DoubleRow

NCORES = 8
N = 8192          # nodes
C = N // NCORES   # own nodes per core (1024)
P = 128           # partitions
KCH = N // P      # node chunks (64)
KP = KCH // 2     # DoubleRow chunk pairs (32)
MB = C // P       # own-node tiles (8)
B = 4             # batch
IT = 18           # I_tot = in_dim + units
FP = B * IT       # packed feature width (72)
FPAD = B * 32     # padded feature width (128)
U = 16            # units
IN_DIM = 2
HC = C // 2       # half own-node width (512)

C0 = 4096.0       # z0 = C0 * d_inv * x0     (fp8-range normalizer)
C1 = 262144.0     # z1 = C1 * d_inv * x1
S0 = 1.0 / C0
S1 = 2.0 / C1

_CACHE = {}


def _build():
    nc = bacc.Bacc("TRN2", target_bir_lowering=False, debug=False,
                   num_devices=NCORES)

    a_d = nc.dram_tensor("a", [P, KCH, C], FP8, kind="ExternalInput")
    z0_d = nc.dram_tensor("z0", [P, KCH, FPAD], FP8, kind="ExternalInput")
    x0T_d = nc.dram_tensor("x0T", [P, C], BF16, kind="ExternalInput")
    hxT_d = nc.dram_tensor("hxT", [P, C], BF16, kind="ExternalInput")
    wruR_d = nc.dram_tensor("wruR", [P, 3, 32], BF16, kind="ExternalInput")
    wruU_d = nc.dram_tensor("wruU", [P, 3, 32], BF16, kind="ExternalInput")
    wc_d = nc.dram_tensor("wc", [P, 3, 32], BF16, kind="ExternalInput")
    brur_d = nc.dram_tensor("brur", [P, 1], F32, kind="ExternalInput")
    bruu_d = nc.dram_tensor("bruu", [P, 1], F32, kind="ExternalInput")
    bc_d = nc.dram_tensor("bc", [P, 1], F32, kind="ExternalInput")
    cdzA_d = nc.dram_tensor("cdzA", [P, MB], BF16, kind="ExternalInput")
    cdzB_d = nc.dram_tensor("cdzB", [P, MB], BF16, kind="ExternalInput")
    ident_d = nc.dram_tensor("ident", [P, P], BF16, kind="ExternalInput")
    out_d = nc.dram_tensor("out", [P, C], F32, kind="ExternalOutput")

    with tile.TileContext(nc) as tc:
        with (
            tc.tile_pool(name="big", bufs=1) as big,
            tc.tile_pool(name="psmm", bufs=2, space="PSUM") as psmm,
            tc.tile_pool(name="pstp", bufs=2, space="PSUM") as pstp,
            tc.tile_pool(name="psg", bufs=4, space="PSUM") as psg,
            tc.tile_pool(name="dram", bufs=1, space="DRAM") as dram,
        ):
            # ---------- persistent SBUF tensors ----------
            NDMA = 8
            CPI = KCH // NDMA  # chunks per stream DMA (8)
            abf = [big.tile([P, CPI, C], FP8, name=f"abf{i}")
                   for i in range(NDMA)]

            z0 = big.tile([P, KCH, FPAD], FP8)
            zgA = big.tile([P, KCH, FPAD], FP8)    # gathered z1 / z1c
            zgB = big.tile([P, KCH, FPAD], FP8)    # gathered z0c
            x0T = big.tile([P, C], BF16)           # becomes x0cT after rh
            hxT = big.tile([P, C], BF16)
            y1raw = big.tile([P, C], BF16)         # c0*x1 own (raw psum)
            x2raw = big.tile([P, C], BF16)         # c1/2*T@x1 own
            y1craw = big.tile([P, C], BF16)
            x2craw = big.tile([P, C], BF16)
            sigR = big.tile([P, C], BF16)
            sigU = big.tile([P, C], BF16)
            cT = big.tile([P, C], BF16)
            outT = big.tile([P, C], F32)
            wruR = big.tile([P, 3, 32], BF16)
            wruU = big.tile([P, 3, 32], BF16)
            wc = big.tile([P, 3, 32], BF16)
            brur = big.tile([P, 1], F32)
            bruu = big.tile([P, 1], F32)
            bc = big.tile([P, 1], F32)
            cdzA = big.tile([P, MB], BF16)
            cdzB = big.tile([P, MB], BF16)
            identbf = big.tile([P, P], BF16)
            pkA = big.tile([P, MB, B, 32], FP8)    # AG payloads (padded,
            pkB = big.tile([P, MB, B, 32], FP8)    #  pre-scaled)
            pkC = big.tile([P, MB, B, 32], FP8)

            # ---------- input DMAs ----------
            nc.sync.dma_start(z0[:], z0_d[:])
            nc.scalar.dma_start(x0T[:], x0T_d[:])
            nc.scalar.dma_start(hxT[:], hxT_d[:])
            nc.gpsimd.dma_start(wruR[:], wruR_d[:])
            nc.gpsimd.dma_start(wruU[:], wruU_d[:])
            nc.gpsimd.dma_start(wc[:], wc_d[:])
            nc.gpsimd.dma_start(brur[:], brur_d[:])
            nc.gpsimd.dma_start(bruu[:], bruu_d[:])
            nc.gpsimd.dma_start(bc[:], bc_d[:])
            nc.gpsimd.dma_start(cdzA[:], cdzA_d[:])
            nc.gpsimd.dma_start(cdzB[:], cdzB_d[:])
            nc.gpsimd.dma_start(identbf[:], ident_d[:])

            # ---------- adjacency stream (fp8, p-major contiguous) ----------
            ENGS = [nc.sync, nc.scalar, nc.gpsimd] * 3
            for i in range(NDMA):
                ENGS[i].dma_start(abf[i][:], a_d[:, i * CPI:(i + 1) * CPI, :])

            def apair(kp, h):
                """Moving operand [128, 2, 512] for chunk pair kp, half h."""
                i, kk = (2 * kp) // CPI, (2 * kp) % CPI
                return abf[i][:, kk:kk + 2, h * HC:(h + 1) * HC]

            def mm_half(ps, z, h):
                for kp in range(KP):
                    nc.tensor.matmul(
                        ps[:], lhsT=z[:, 2 * kp:2 * kp + 2, :],
                        rhs=apair(kp, h),
                        start=(kp == 0), stop=(kp == KP - 1),
                        perf_mode=DR,
                    )

            def transposes(srcT, pk, cdz, mbs):
                """srcT node-block -> node-major padded, scaled by cdz."""
                for mb in mbs:
                    pt = pstp.tile([P, P], BF16, tag="tp")
                    nc.tensor.transpose(
                        pt[:], srcT[:, mb * P:(mb + 1) * P], identbf[:])
                    cdb = cdz[:, mb:mb + 1].unsqueeze(-1).broadcast_to(
                        (P, B, 32))
                    nc.vector.tensor_tensor(
                        pk[:, mb, :, :],
                        pt[:].rearrange("p (b e) -> p b e", b=B),
                        cdb, ALU.mult)

            def allgather(pk, zdst):
                agin = dram.tile([P, MB, B, 32], FP8, tag="agin")
                agout = dram.tile([NCORES, P, MB, B, 32], FP8,
                                  addr_space="Shared", tag="agout")
                nc.sync.dma_start(agin[:], pk[:])
                nc.gpsimd.collective_compute(
                    "AllGather", ALU.bypass,
                    replica_groups=[list(range(NCORES))],
                    ins=[agin[:]], outs=[agout[:]],
                )
                # gather lands directly in the z tile, split across queues
                for eng, r0, r1 in ((nc.sync, 0, 3), (nc.scalar, 3, 6),
                                    (nc.gpsimd, 6, 8)):
                    eng.dma_start(
                        zdst[:, r0 * MB:r1 * MB, :].rearrange(
                            "p (r m) f -> p r (m f)", r=r1 - r0),
                        agout[r0:r1].rearrange("r p m b e -> p r (m b e)"),
                    )

            def gate_m01(wbf, srcs01, gname):
                """Open gate psum groups with the m=0,1 terms (early)."""
                halves = []
                for h in range(2):
                    pg = psg.tile([P, HC], F32, tag="gate",
                                  name=f"pg{gname}{h}", bufs=4)
                    for b in range(B):
                        for m in range(2):
                            nc.tensor.matmul(
                                pg[b * 32:(b + 1) * 32, :],
                                lhsT=wbf[b * 32:b * 32 + IT, m, :],
                                rhs=srcs01[m][b * 32:b * 32 + IT,
                                              h * HC:(h + 1) * HC],
                                start=(m == 0), stop=False,
                                tile_position=(b * 32, b * 32),
                            )
                    halves.append(pg)
                return halves

            def gate_m2(pg, wbf, src2, h):
                for b in range(B):
                    nc.tensor.matmul(
                        pg[b * 32:(b + 1) * 32, :],
                        lhsT=wbf[b * 32:b * 32 + IT, 2, :],
                        rhs=src2[b * 32:b * 32 + IT, h * HC:(h + 1) * HC],
                        start=False, stop=True,
                        tile_position=(b * 32, b * 32),
                    )

            # ================= gconv 1 (r/u gates) =================
            # diff A chases the stream (h-inner)
            psA = [psmm.tile([P, HC], F32, tag="mm", name=f"psA{h}")
                   for h in range(2)]
            for kp in range(KP):
                for h in range(2):
                    nc.tensor.matmul(
                        psA[h][:], lhsT=z0[:, 2 * kp:2 * kp + 2, :],
                        rhs=apair(kp, h),
                        start=(kp == 0), stop=(kp == KP - 1),
                        perf_mode=DR,
                    )
            for h in range(2):
                nc.vector.tensor_copy(y1raw[:, h * HC:(h + 1) * HC],
                                      psA[h][:])
            transposes(y1raw, pkA, cdzA, range(MB))
            allgather(pkA, zgA)
            # r/u gate m=0,1 run inside the AG window
            pg_r = gate_m01(wruR, [x0T, y1raw], "r")
            pg_u = gate_m01(wruU, [x0T, y1raw], "u")

            # diff B (h-outer, per-half tail)
            psB = [psmm.tile([P, HC], F32, tag="mm", name=f"psB{h}")
                   for h in range(2)]
            for h in range(2):
                fs = slice(h * HC, (h + 1) * HC)
                mm_half(psB[h], zgA, h)
                nc.vector.tensor_copy(x2raw[:, fs], psB[h][:])
                gate_m2(pg_r[h], wruR, x2raw, h)
                gate_m2(pg_u[h], wruU, x2raw, h)
                nc.scalar.activation(sigR[:, fs], pg_r[h][:], AF.Sigmoid,
                                     bias=brur[:])
                for b in range(B):
                    nc.vector.tensor_tensor(
                        x0T[b * 32:b * 32 + U, fs],
                        sigR[b * 32:b * 32 + U, fs],
                        hxT[b * 32:b * 32 + U, fs],
                        ALU.mult,
                    )
                transposes(x0T, pkB, cdzB, range(h * MB // 2,
                                                 (h + 1) * MB // 2))
            allgather(pkB, zgB)
            for h in range(2):
                nc.scalar.activation(sigU[:, h * HC:(h + 1) * HC],
                                     pg_u[h][:], AF.Sigmoid, bias=bruu[:])

            # ================= gconv 2 (candidate c) =================
            psC = [psmm.tile([P, HC], F32, tag="mm", name=f"psC{h}")
                   for h in range(2)]
            for h in range(2):
                fs = slice(h * HC, (h + 1) * HC)
                mm_half(psC[h], zgB, h)
                nc.vector.tensor_copy(y1craw[:, fs], psC[h][:])
                transposes(y1craw, pkC, cdzA, range(h * MB // 2,
                                                    (h + 1) * MB // 2))
            allgather(pkC, zgA)
            pg_c = gate_m01(wc, [x0T, y1craw], "c")

            # diff D + per-half tail to the output DMA
            psD = [psmm.tile([P, HC], F32, tag="mm", name=f"psD{h}")
                   for h in range(2)]
            for h in range(2):
                fs = slice(h * HC, (h + 1) * HC)
                mm_half(psD[h], zgA, h)
                nc.vector.tensor_copy(x2craw[:, fs], psD[h][:])
                gate_m2(pg_c[h], wc, x2craw, h)
                nc.scalar.activation(cT[:, fs], pg_c[h][:], AF.Tanh,
                                     bias=bc[:])
                # out = c + u*(h - c)
                eng = nc.gpsimd if h == 0 else nc.vector
                eng.tensor_tensor(outT[:, fs], hxT[:, fs], cT[:, fs],
                                  ALU.subtract)
                eng.tensor_tensor(outT[:, fs], outT[:, fs], sigU[:, fs],
                                  ALU.mult)
                eng.tensor_tensor(outT[:, fs], outT[:, fs], cT[:, fs],
                                  ALU.add)
                (nc.sync if h == 0 else nc.scalar).dma_start(
                    out_d[:, fs], outT[:, fs])

    nc.compile()
    return nc


def _get_nc():
    if "nc" not in _CACHE:
        _CACHE["nc"] = _build()
    return _CACHE["nc"]


# feature permutation: device feature j -> reference feature i
# j = 0..15 -> i = j+2 (hidden), j = 16,17 -> i = j-16 (input x)
_PERM = np.array(list(range(2, 18)) + [0, 1])


def _host_prep(inputs, hx, adj, W_ru, b_ru, W_c, b_c):
    f32 = np.float32
    bf16 = ml_dtypes.bfloat16
    fp8 = ml_dtypes.float8_e4m3fn

    xr = np.ascontiguousarray(inputs, dtype=f32).reshape(B, N, IN_DIM)
    hr = np.ascontiguousarray(hx, dtype=f32).reshape(B, N, U)
    x0 = np.concatenate([hr, xr], axis=2).transpose(1, 0, 2)  # [N, B, 18]
    x0 = np.ascontiguousarray(x0).reshape(N, FP)

    adj = np.asarray(adj, f32)
    d = adj.sum(axis=1) + 1.0
    dinv = 1.0 / d

    # z0 = C0 * dinv * x0, padded to 32 cols per batch, p-major, fp8
    z0 = np.zeros((N, B, 32), f32)
    z0[:, :, 0:IT] = (C0 * dinv)[:, None, None] * x0.reshape(N, B, IT)
    z0 = z0.reshape(KCH, P, FPAD).transpose(1, 0, 2)  # [P, KCH, FPAD]
    z0 = np.ascontiguousarray(z0).astype(fp8)

    def pad_w(w, lo):
        # fold the diffusion unscales into the weights:
        # gate = x0*(W0-W2) + (c0*x1)*(W1/c0) + (c1/2*Tx1)*(2*W2/c1)
        w3 = np.asarray(w, f32).reshape(IT, 3, -1)[_PERM][:, :, lo:lo + U]
        w3 = np.stack([w3[:, 0] - w3[:, 2], S0 * w3[:, 1], S1 * w3[:, 2]],
                      axis=1)
        out = np.zeros((B, 32, 3, 32), f32)
        out[:, 0:IT, :, 0:U] = w3[None]
        return out.reshape(P, 3, 32).astype(bf16)

    wruR_p = pad_w(W_ru, 0)
    wruU_p = pad_w(W_ru, U)
    wc_p = pad_w(W_c, 0)

    def pad_bias(v):
        t = np.zeros((B, 32), f32)
        t[:, 0:U] = np.asarray(v, f32)
        return np.ascontiguousarray(t.reshape(P)[:, None])

    brur_t = pad_bias(np.asarray(b_ru, f32)[0:U])
    bruu_t = pad_bias(np.asarray(b_ru, f32)[U:2 * U])
    bc_t = pad_bias(np.asarray(b_c, f32))
    ident = np.eye(P, dtype=f32).astype(bf16)

    in_maps = []
    for m in range(NCORES):
        sl = slice(m * C, (m + 1) * C)
        a_m = np.ascontiguousarray(adj[:, sl])
        a_m[m * C + np.arange(C), np.arange(C)] += 1.0
        # p-major: [p, k, c] = row k*128+p
        a_m = a_m.reshape(KCH, P, C).transpose(1, 0, 2)
        a_m = np.ascontiguousarray(a_m).astype(fp8)

        x0own = x0[sl]
        x0T = np.zeros((B, 32, C), f32)
        x0T[:, 0:IT, :] = x0own.reshape(C, B, IT).transpose(1, 2, 0)
        x0T = x0T.reshape(P, C).astype(bf16)
        hxT_p = np.zeros((B, 32, C), f32)
        hxT_p[:, 0:U, :] = hr[:, sl, :].transpose(0, 2, 1)
        hxT_p = hxT_p.reshape(P, C).astype(bf16)
        # per-own-node payload scales, node-major [p, mb]
        dlocal = dinv[sl].reshape(MB, P).T
        cdzA_p = np.ascontiguousarray((C1 / C0) * dlocal).astype(bf16)
        cdzB_p = np.ascontiguousarray(C0 * dlocal).astype(bf16)
        in_maps.append({
            "a": a_m,
            "z0": z0,
            "x0T": x0T,
            "hxT": hxT_p,
            "wruR": wruR_p,
            "wruU": wruU_p,
            "wc": wc_p,
            "brur": brur_t,
            "bruu": bruu_t,
            "bc": bc_t,
            "cdzA": cdzA_p,
            "cdzB": cdzB_p,
            "ident": ident,
        })
    return in_maps


def _run(in_maps, trace=False, **kw):
    nc = _get_nc()
    return bass_utils.run_bass_kernel_spmd(
        nc, in_maps, core_ids=list(range(NCORES)), trace=trace, **kw)


def _assemble(results):
    out = np.empty((B, N * U), np.float32)
    for m in range(NCORES):
        # device layout [b*32+u, n] (rows 16..31 per block are padding)
        blk = results[m]["out"].reshape(B, 32, C)[:, 0:U, :].transpose(0, 2, 1)
        out[:, m * C * U:(m + 1) * C * U] = blk.reshape(B, C * U)
    return out


def kernel(inputs, hx, adj, W_ru, b_ru, W_c, b_c):
    in_maps = _host_prep(inputs, hx, adj, W_ru, b_ru, W_c, b_c)
    res = _run(in_maps)
    return _assemble(res.results)


# revision 9
# speedup vs baseline: 1.6779x; 1.0118x over previous
"""DCGRU cell on 8 Trainium2 NeuronCores (Bass/Tile), v3.

Math: with a = adj + I, d = a.sum(axis=1), T = (d^-1 a)^T, every
diffusion step is  y = T @ v = a^T @ (d_inv * v).  d_inv is computed on
the HOST, so there are no row-sum collectives.  The d_inv factor rides
on the activation side: the stationary operand of each diffusion matmul
is z = c * d_inv * v (c a power of 2 keeping fp8 values in normal
range).  All unscale constants fold into the host-prepared gate weights
(W0' = W0 - W2, W1' = W1/c0, W2' = 2*W2/c1), so diffusion PSUMs are
evacuated as raw bf16 copies and the Chebyshev combine x2 = 2*T@x1 - x0
never materializes.

Sharding (8 cores): 1D column-parallel over the adjacency.  Core m
holds a[:, m*1024:(m+1)*1024] as fp8e4 (8 MB), host-permuted p-major so
partition lines are contiguous.  Each diffusion is a DoubleRow fp8
matmul: stationary z pair-chunk [128, 2, 128], moving adjacency
[128, 2, 512] -> psum [128, 512]; output is feature-major [b*32+j, n]
which is what the gate matmuls consume directly.  AllGather payloads
are node-major padded fp8, pre-scaled by c*d_inv during the
transpose-evacuation, so the gather DMA writes the next diffusion's
stationary operand directly (no on-device z build at all).

Feature order is h-first: j = 0..15 hidden, j = 16,17 input; padded to
32 rows/cols per batch so partition bases stay 0/32/64/96.
"""

import numpy as np
import ml_dtypes

import concourse.bass as bass
import concourse.bacc as bacc
import concourse.tile as tile
import concourse.mybir as mybir
import concourse.bass_utils as bass_utils

F32 = mybir.dt.float32
BF16 = mybir.dt.bfloat16
FP8 = mybir.dt.float8e4
AF = mybir.ActivationFunctionType
ALU = mybir.AluOpType
DR = mybir.MatmulPerfMode.DoubleRow

NCORES = 8
N = 8192          # nodes
C = N // NCORES   # own nodes per core (1024)
P = 128           # partitions
KCH = N // P      # node chunks (64)
KP = KCH // 2     # DoubleRow chunk pairs (32)
MB = C // P       # own-node tiles (8)
B = 4             # batch
IT = 18           # I_tot = in_dim + units
FP = B * IT       # packed feature width (72)
FPAD = B * 32     # padded feature width (128)
U = 16            # units
IN_DIM = 2
HC = C // 2       # half own-node width (512)

C0 = 4096.0       # z0 = C0 * d_inv * x0     (fp8-range normalizer)
C1 = 262144.0     # z1 = C1 * d_inv * x1
S0 = 1.0 / C0
S1 = 2.0 / C1

_CACHE = {}


def _build():
    nc = bacc.Bacc("TRN2", target_bir_lowering=False, debug=False,
                   num_devices=NCORES)

    a_d = nc.dram_tensor("a", [P, KCH, C], FP8, kind="ExternalInput")
    z0_d = nc.dram_tensor("z0", [P, KCH, FPAD], FP8, kind="ExternalInput")
    x0T_d = nc.dram_tensor("x0T", [P, C], BF16, kind="ExternalInput")
    hxT_d = nc.dram_tensor("hxT", [P, C], BF16, kind="ExternalInput")
    wruR_d = nc.dram_tensor("wruR", [P, 3, 32], BF16, kind="ExternalInput")
    wruU_d = nc.dram_tensor("wruU", [P, 3, 32], BF16, kind="ExternalInput")
    wc_d = nc.dram_tensor("wc", [P, 3, 32], BF16, kind="ExternalInput")
    brur_d = nc.dram_tensor("brur", [P, 1], F32, kind="ExternalInput")
    bruu_d = nc.dram_tensor("bruu", [P, 1], F32, kind="ExternalInput")
    bc_d = nc.dram_tensor("bc", [P, 1], F32, kind="ExternalInput")
    cdzA_d = nc.dram_tensor("cdzA", [P, MB], BF16, kind="ExternalInput")
    cdzB_d = nc.dram_tensor("cdzB", [P, MB], BF16, kind="ExternalInput")
    ident_d = nc.dram_tensor("ident", [P, P], BF16, kind="ExternalInput")
    out_d = nc.dram_tensor("out", [P, C], F32, kind="ExternalOutput")

    with tile.TileContext(nc) as tc:
        with (
            tc.tile_pool(name="big", bufs=1) as big,
            tc.tile_pool(name="psmm", bufs=2, space="PSUM") as psmm,
            tc.tile_pool(name="pstp", bufs=2, space="PSUM") as pstp,
            tc.tile_pool(name="psg", bufs=4, space="PSUM") as psg,
            tc.tile_pool(name="dram", bufs=1, space="DRAM") as dram,
        ):
            # ---------- persistent SBUF tensors ----------
            NDMA = 16
            CPI = KCH // NDMA  # chunks per stream DMA (4)
            abf = [big.tile([P, CPI, C], FP8, name=f"abf{i}")
                   for i in range(NDMA)]

            z0 = big.tile([P, KCH, FPAD], FP8)
            zgA = big.tile([P, KCH, FPAD], FP8)    # gathered z1 / z1c
            zgB = big.tile([P, KCH, FPAD], FP8)    # gathered z0c
            x0T = big.tile([P, C], BF16)           # becomes x0cT after rh
            hxT = big.tile([P, C], BF16)
            y1raw = big.tile([P, C], BF16)         # c0*x1 own (raw psum)
            x2raw = big.tile([P, C], BF16)         # c1/2*T@x1 own
            y1craw = big.tile([P, C], BF16)
            x2craw = big.tile([P, C], BF16)
            sigR = big.tile([P, C], BF16)
            sigU = big.tile([P, C], BF16)
            cT = big.tile([P, C], BF16)
            outT = big.tile([P, C], F32)
            wruR = big.tile([P, 3, 32], BF16)
            wruU = big.tile([P, 3, 32], BF16)
            wc = big.tile([P, 3, 32], BF16)
            brur = big.tile([P, 1], F32)
            bruu = big.tile([P, 1], F32)
            bc = big.tile([P, 1], F32)
            cdzA = big.tile([P, MB], BF16)
            cdzB = big.tile([P, MB], BF16)
            identbf = big.tile([P, P], BF16)
            pkA = big.tile([P, MB, B, 32], FP8)    # AG payloads (padded,
            pkB = big.tile([P, MB, B, 32], FP8)    #  pre-scaled)
            pkC = big.tile([P, MB, B, 32], FP8)

            # ---------- input DMAs ----------
            nc.scalar.dma_start(z0[:], z0_d[:])
            nc.scalar.dma_start(x0T[:], x0T_d[:])
            nc.scalar.dma_start(hxT[:], hxT_d[:])
            nc.gpsimd.dma_start(wruR[:], wruR_d[:])
            nc.gpsimd.dma_start(wruU[:], wruU_d[:])
            nc.gpsimd.dma_start(wc[:], wc_d[:])
            nc.gpsimd.dma_start(brur[:], brur_d[:])
            nc.gpsimd.dma_start(bruu[:], bruu_d[:])
            nc.gpsimd.dma_start(bc[:], bc_d[:])
            nc.gpsimd.dma_start(cdzA[:], cdzA_d[:])
            nc.gpsimd.dma_start(cdzB[:], cdzB_d[:])
            nc.gpsimd.dma_start(identbf[:], ident_d[:])

            # ---------- adjacency stream (fp8, p-major contiguous) ----------
            # scalar carries z0/x0T/hxT (1.5 MB) so it gets fewer tiles;
            # queue order interleaves so chunks arrive roughly in kp order
            S, G, Csc = nc.sync, nc.gpsimd, nc.scalar
            ENGS = [S, G, Csc, S, G, S, G, Csc, S, G, S, Csc, G, S, G, Csc]
            for i in range(NDMA):
                ENGS[i].dma_start(abf[i][:], a_d[:, i * CPI:(i + 1) * CPI, :])

            def apair(kp, h):
                """Moving operand [128, 2, 512] for chunk pair kp, half h."""
                i, kk = (2 * kp) // CPI, (2 * kp) % CPI
                return abf[i][:, kk:kk + 2, h * HC:(h + 1) * HC]

            def mm_half(ps, z, h):
                for kp in range(KP):
                    nc.tensor.matmul(
                        ps[:], lhsT=z[:, 2 * kp:2 * kp + 2, :],
                        rhs=apair(kp, h),
                        start=(kp == 0), stop=(kp == KP - 1),
                        perf_mode=DR,
                    )

            def transposes(srcT, pk, cdz, mbs):
                """srcT node-block -> node-major padded, scaled by cdz."""
                for mb in mbs:
                    pt = pstp.tile([P, P], BF16, tag="tp")
                    nc.tensor.transpose(
                        pt[:], srcT[:, mb * P:(mb + 1) * P], identbf[:])
                    cdb = cdz[:, mb:mb + 1].unsqueeze(-1).broadcast_to(
                        (P, B, 32))
                    nc.vector.tensor_tensor(
                        pk[:, mb, :, :],
                        pt[:].rearrange("p (b e) -> p b e", b=B),
                        cdb, ALU.mult)

            def allgather(pk, zdst):
                agin = dram.tile([P, MB, B, 32], FP8, tag="agin")
                agout = dram.tile([NCORES, P, MB, B, 32], FP8,
                                  addr_space="Shared", tag="agout")
                # gpsimd: its stream share drains early, so the payload
                # DMA + trigger don't queue behind stream traffic
                nc.gpsimd.dma_start(agin[:], pk[:])
                nc.gpsimd.collective_compute(
                    "AllGather", ALU.bypass,
                    replica_groups=[list(range(NCORES))],
                    ins=[agin[:]], outs=[agout[:]],
                )
                # gather lands directly in the z tile, split across queues
                for eng, r0, r1 in ((nc.sync, 0, 3), (nc.scalar, 3, 6),
                                    (nc.gpsimd, 6, 8)):
                    eng.dma_start(
                        zdst[:, r0 * MB:r1 * MB, :].rearrange(
                            "p (r m) f -> p r (m f)", r=r1 - r0),
                        agout[r0:r1].rearrange("r p m b e -> p r (m b e)"),
                    )

            def gate_m01(wbf, srcs01, gname):
                """Open gate psum groups with the m=0,1 terms (early)."""
                halves = []
                for h in range(2):
                    pg = psg.tile([P, HC], F32, tag="gate",
                                  name=f"pg{gname}{h}", bufs=4)
                    for b in range(B):
                        for m in range(2):
                            nc.tensor.matmul(
                                pg[b * 32:(b + 1) * 32, :],
                                lhsT=wbf[b * 32:b * 32 + IT, m, :],
                                rhs=srcs01[m][b * 32:b * 32 + IT,
                                              h * HC:(h + 1) * HC],
                                start=(m == 0), stop=False,
                                tile_position=(b * 32, b * 32),
                            )
                    halves.append(pg)
                return halves

            def gate_m2(pg, wbf, src2, h):
                for b in range(B):
                    nc.tensor.matmul(
                        pg[b * 32:(b + 1) * 32, :],
                        lhsT=wbf[b * 32:b * 32 + IT, 2, :],
                        rhs=src2[b * 32:b * 32 + IT, h * HC:(h + 1) * HC],
                        start=False, stop=True,
                        tile_position=(b * 32, b * 32),
                    )

            # ================= gconv 1 (r/u gates) =================
            # diff A chases the stream (h-inner)
            psA = [psmm.tile([P, HC], F32, tag="mm", name=f"psA{h}")
                   for h in range(2)]
            for kp in range(KP):
                for h in range(2):
                    nc.tensor.matmul(
                        psA[h][:], lhsT=z0[:, 2 * kp:2 * kp + 2, :],
                        rhs=apair(kp, h),
                        start=(kp == 0), stop=(kp == KP - 1),
                        perf_mode=DR,
                    )
            for h in range(2):
                nc.vector.tensor_copy(y1raw[:, h * HC:(h + 1) * HC],
                                      psA[h][:])
            transposes(y1raw, pkA, cdzA, range(MB))
            allgather(pkA, zgA)
            # r/u gate m=0,1 run inside the AG window
            pg_r = gate_m01(wruR, [x0T, y1raw], "r")
            pg_u = gate_m01(wruU, [x0T, y1raw], "u")

            # diff B (h-outer, per-half tail)
            psB = [psmm.tile([P, HC], F32, tag="mm", name=f"psB{h}")
                   for h in range(2)]
            for h in range(2):
                fs = slice(h * HC, (h + 1) * HC)
                mm_half(psB[h], zgA, h)
                nc.vector.tensor_copy(x2raw[:, fs], psB[h][:])
                gate_m2(pg_r[h], wruR, x2raw, h)
                gate_m2(pg_u[h], wruU, x2raw, h)
                nc.scalar.activation(sigR[:, fs], pg_r[h][:], AF.Sigmoid,
                                     bias=brur[:])
                for b in range(B):
                    nc.vector.tensor_tensor(
                        x0T[b * 32:b * 32 + U, fs],
                        sigR[b * 32:b * 32 + U, fs],
                        hxT[b * 32:b * 32 + U, fs],
                        ALU.mult,
                    )
                transposes(x0T, pkB, cdzB, range(h * MB // 2,
                                                 (h + 1) * MB // 2))
            allgather(pkB, zgB)
            for h in range(2):
                nc.scalar.activation(sigU[:, h * HC:(h + 1) * HC],
                                     pg_u[h][:], AF.Sigmoid, bias=bruu[:])

            # ================= gconv 2 (candidate c) =================
            psC = [psmm.tile([P, HC], F32, tag="mm", name=f"psC{h}")
                   for h in range(2)]
            for h in range(2):
                fs = slice(h * HC, (h + 1) * HC)
                mm_half(psC[h], zgB, h)
                nc.vector.tensor_copy(y1craw[:, fs], psC[h][:])
                transposes(y1craw, pkC, cdzA, range(h * MB // 2,
                                                    (h + 1) * MB // 2))
            allgather(pkC, zgA)
            pg_c = gate_m01(wc, [x0T, y1craw], "c")

            # diff D + per-half tail to the output DMA
            psD = [psmm.tile([P, HC], F32, tag="mm", name=f"psD{h}")
                   for h in range(2)]
            for h in range(2):
                fs = slice(h * HC, (h + 1) * HC)
                mm_half(psD[h], zgA, h)
                nc.vector.tensor_copy(x2craw[:, fs], psD[h][:])
                gate_m2(pg_c[h], wc, x2craw, h)
                nc.scalar.activation(cT[:, fs], pg_c[h][:], AF.Tanh,
                                     bias=bc[:])
                # out = c + u*(h - c)
                eng = nc.gpsimd if h == 0 else nc.vector
                eng.tensor_tensor(outT[:, fs], hxT[:, fs], cT[:, fs],
                                  ALU.subtract)
                eng.tensor_tensor(outT[:, fs], outT[:, fs], sigU[:, fs],
                                  ALU.mult)
                eng.tensor_tensor(outT[:, fs], outT[:, fs], cT[:, fs],
                                  ALU.add)
                (nc.sync if h == 0 else nc.scalar).dma_start(
                    out_d[:, fs], outT[:, fs])

    nc.compile()
    return nc


def _get_nc():
    if "nc" not in _CACHE:
        _CACHE["nc"] = _build()
    return _CACHE["nc"]


# feature permutation: device feature j -> reference feature i
# j = 0..15 -> i = j+2 (hidden), j = 16,17 -> i = j-16 (input x)
_PERM = np.array(list(range(2, 18)) + [0, 1])


def _host_prep(inputs, hx, adj, W_ru, b_ru, W_c, b_c):
    f32 = np.float32
    bf16 = ml_dtypes.bfloat16
    fp8 = ml_dtypes.float8_e4m3fn

    xr = np.ascontiguousarray(inputs, dtype=f32).reshape(B, N, IN_DIM)
    hr = np.ascontiguousarray(hx, dtype=f32).reshape(B, N, U)
    x0 = np.concatenate([hr, xr], axis=2).transpose(1, 0, 2)  # [N, B, 18]
    x0 = np.ascontiguousarray(x0).reshape(N, FP)

    adj = np.asarray(adj, f32)
    d = adj.sum(axis=1) + 1.0
    dinv = 1.0 / d

    # z0 = C0 * dinv * x0, padded to 32 cols per batch, p-major, fp8
    z0 = np.zeros((N, B, 32), f32)
    z0[:, :, 0:IT] = (C0 * dinv)[:, None, None] * x0.reshape(N, B, IT)
    z0 = z0.reshape(KCH, P, FPAD).transpose(1, 0, 2)  # [P, KCH, FPAD]
    z0 = np.ascontiguousarray(z0).astype(fp8)

    def pad_w(w, lo):
        # fold the diffusion unscales into the weights:
        # gate = x0*(W0-W2) + (c0*x1)*(W1/c0) + (c1/2*Tx1)*(2*W2/c1)
        w3 = np.asarray(w, f32).reshape(IT, 3, -1)[_PERM][:, :, lo:lo + U]
        w3 = np.stack([w3[:, 0] - w3[:, 2], S0 * w3[:, 1], S1 * w3[:, 2]],
                      axis=1)
        out = np.zeros((B, 32, 3, 32), f32)
        out[:, 0:IT, :, 0:U] = w3[None]
        return out.reshape(P, 3, 32).astype(bf16)

    wruR_p = pad_w(W_ru, 0)
    wruU_p = pad_w(W_ru, U)
    wc_p = pad_w(W_c, 0)

    def pad_bias(v):
        t = np.zeros((B, 32), f32)
        t[:, 0:U] = np.asarray(v, f32)
        return np.ascontiguousarray(t.reshape(P)[:, None])

    brur_t = pad_bias(np.asarray(b_ru, f32)[0:U])
    bruu_t = pad_bias(np.asarray(b_ru, f32)[U:2 * U])
    bc_t = pad_bias(np.asarray(b_c, f32))
    ident = np.eye(P, dtype=f32).astype(bf16)

    in_maps = []
    for m in range(NCORES):
        sl = slice(m * C, (m + 1) * C)
        a_m = np.ascontiguousarray(adj[:, sl])
        a_m[m * C + np.arange(C), np.arange(C)] += 1.0
        # p-major: [p, k, c] = row k*128+p
        a_m = a_m.reshape(KCH, P, C).transpose(1, 0, 2)
        a_m = np.ascontiguousarray(a_m).astype(fp8)

        x0own = x0[sl]
        x0T = np.zeros((B, 32, C), f32)
        x0T[:, 0:IT, :] = x0own.reshape(C, B, IT).transpose(1, 2, 0)
        x0T = x0T.reshape(P, C).astype(bf16)
        hxT_p = np.zeros((B, 32, C), f32)
        hxT_p[:, 0:U, :] = hr[:, sl, :].transpose(0, 2, 1)
        hxT_p = hxT_p.reshape(P, C).astype(bf16)
        # per-own-node payload scales, node-major [p, mb]
        dlocal = dinv[sl].reshape(MB, P).T
        cdzA_p = np.ascontiguousarray((C1 / C0) * dlocal).astype(bf16)
        cdzB_p = np.ascontiguousarray(C0 * dlocal).astype(bf16)
        in_maps.append({
            "a": a_m,
            "z0": z0,
            "x0T": x0T,
            "hxT": hxT_p,
            "wruR": wruR_p,
            "wruU": wruU_p,
            "wc": wc_p,
            "brur": brur_t,
            "bruu": bruu_t,
            "bc": bc_t,
            "cdzA": cdzA_p,
            "cdzB": cdzB_p,
            "ident": ident,
        })
    return in_maps


def _run(in_maps, trace=False, **kw):
    nc = _get_nc()
    return bass_utils.run_bass_kernel_spmd(
        nc, in_maps, core_ids=list(range(NCORES)), trace=trace, **kw)


def _assemble(results):
    out = np.empty((B, N * U), np.float32)
    for m in range(NCORES):
        # device layout [b*32+u, n] (rows 16..31 per block are padding)
        blk = results[m]["out"].reshape(B, 32, C)[:, 0:U, :].transpose(0, 2, 1)
        out[:, m * C * U:(m + 1) * C * U] = blk.reshape(B, C * U)
    return out


def kernel(inputs, hx, adj, W_ru, b_ru, W_c, b_c):
    in_maps = _host_prep(inputs, hx, adj, W_ru, b_ru, W_c, b_c)
    res = _run(in_maps)
    return _assemble(res.results)
